# revision 12
# baseline (speedup 1.0000x reference)
"""Trainium2 Bass kernel for cosine-similarity hint attention.

Computation (per batch b):
  sp = state_emb @ Ws.T + bs                  (B, A)
  hp = hints_emb @ Wh.T + bh                  (B, N, A)
  scores = <sp, hp> / (max(|sp|,eps) * max(|hp|,eps))
  attn = softmax(scores, axis=N)
  out = attn @ hints_emb                      (B, HD)

Distribution: data-parallel over batch, B=512 -> 64 per core on 8 cores.
Weights replicated. No collectives.

Device-side algorithm (per core, per group of 8 batches = 16 row-tiles of
128 hint-rows):
  - hints arrive in two host-prepared bf16 layouts: natural [r, h] tiles
    (for the final weighted sum, contraction over rows) and transposed
    [h, r] tiles (for the hint projection, contraction over h).
  - hint projection z = X @ Wh.T is computed on TensorE with an augmented
    moving operand [Wh.T | q_b | wb] where q_b = sp_b @ Wh and
    wb = Wh.T @ bh. This yields, per row r: z (256 cols),
    zq = <sp_b, z_r> (col 256), zw = <z_r, bh> (col 257) in one pass.
  - |hp_r|^2 = sum(z_r^2) + 2*zw + |bh|^2, sum of squares via one
    ScalarE Square+accumulate pass over PSUM.
  - scores = (zq + <sp,bh>) * rsn_b * rhn_r, then a [128,8] PE transpose
    puts each batch's 256 scores on one partition for the softmax.
  - weighted sum uses a block-diagonal attn stationary [128, 8] so all 8
    batches of a group accumulate in one PSUM [8, 512] tile.
"""

import sys

if "/opt/trn_rl_repo" not in sys.path:
    sys.path.insert(0, "/opt/trn_rl_repo")

import numpy as np
import ml_dtypes

import concourse.bass as bass
import concourse.mybir as mybir
import concourse.tile as tile
from concourse import bacc
from concourse.masks import make_identity
from concourse.bass_utils import run_bass_kernel_spmd

# Problem shapes (hardcoded per harness contract)
B, N, SD, HD, AD = 512, 256, 1024, 512, 256
NCORES = 8
BL = B // NCORES          # 64 batches per core
G = 8                     # batches per group
NG = BL // G              # 8 groups
TPG = G * N // 128        # 16 row-tiles (128 hint-rows) per group
KH = HD // 128            # 4 contraction chunks over HD
EPS = 1e-8

F32 = mybir.dt.float32
BF16 = mybir.dt.bfloat16
AF = mybir.ActivationFunctionType
ALU = mybir.AluOpType
AX = mybir.AxisListType


def build_nc(stage="full"):
    """stage: 'p1'..'p12' stop after that prologue step; 'prologue';
    'proj'; 'scores'; 'softmax'; 'full'."""
    if stage.startswith("p") and stage[1:].isdigit():
        cut = int(stage[1:])
    else:
        cut = 99

    nc = bacc.Bacc("TRN2", target_bir_lowering=False, debug=False,
                   num_devices=NCORES)

    state = nc.dram_tensor("state", [BL, SD], F32, kind="ExternalInput")
    xnat = nc.dram_tensor("xnat", [NG, 128, TPG, 512], BF16,
                          kind="ExternalInput")
    xt = nc.dram_tensor("xt", [NG, 128, TPG, KH, 128], BF16,
                        kind="ExternalInput")
    ws2 = nc.dram_tensor("ws2", [128, 2, SD], F32, kind="ExternalInput")
    bsb = nc.dram_tensor("bsb", [BL, AD], F32, kind="ExternalInput")
    wh2 = nc.dram_tensor("wh2", [128, 2, HD], F32, kind="ExternalInput")
    bh2 = nc.dram_tensor("bh2", [128, 2], F32, kind="ExternalInput")
    bhb = nc.dram_tensor("bhb", [BL, AD], F32, kind="ExternalInput")
    out = nc.dram_tensor("out", [BL, HD], F32, kind="ExternalOutput")

    with tile.TileContext(nc) as tc:
        with (
            tc.tile_pool(name="singles", bufs=1) as singles,
            tc.tile_pool(name="xpool", bufs=2) as xpool,
            tc.tile_pool(name="work", bufs=2) as work,
            tc.tile_pool(name="scratch", bufs=2) as scratch,
            tc.tile_pool(name="dram", bufs=1, space="DRAM") as dram,
            tc.tile_pool(name="psz", bufs=3, space="PSUM") as psz_pool,
            tc.tile_pool(name="pss", bufs=2, space="PSUM") as pss_pool,
            tc.tile_pool(name="psw", bufs=2, space="PSUM") as psw_pool,
        ):
            # ---------------- prologue ----------------
            ident = singles.tile([128, 128], F32)
            make_identity(nc, ident)

            # load small tensors
            state_sb = singles.tile([BL, SD], F32)
            nc.sync.dma_start(out=state_sb[:], in_=state[:])
            ws2_sb = singles.tile([128, 2, SD], F32)
            nc.sync.dma_start(out=ws2_sb[:], in_=ws2[:])
            wh2_sb = singles.tile([128, 2, HD], F32)
            nc.sync.dma_start(out=wh2_sb[:], in_=wh2[:])
            bh2_sb = singles.tile([128, 2], F32)
            nc.sync.dma_start(out=bh2_sb[:], in_=bh2[:])
            bsb_sb = singles.tile([BL, AD], F32)
            nc.sync.dma_start(out=bsb_sb[:], in_=bsb[:])
            bhb_sb = singles.tile([BL, AD], F32)
            nc.sync.dma_start(out=bhb_sb[:], in_=bhb[:])

            def _prologue():
                # step 1: state.T : [128, 8, 64]
                stateT = singles.tile([128, SD // 128, BL], F32)
                for k in range(SD // 128):
                    pst = pss_pool.tile([128, BL], F32, tag="pt",
                                        name=f"pt_st{k}")
                    nc.tensor.transpose(
                        pst, state_sb[:, k * 128:(k + 1) * 128],
                        ident[:BL, :BL])
                    nc.vector.tensor_copy(out=stateT[:, k, :], in_=pst)
                if cut < 2:
                    return None

                # step 2: Ws.T : [128, 8, 256]
                wsT = singles.tile([128, SD // 128, AD], F32)
                for k in range(SD // 128):
                    for c in range(2):
                        pst = pss_pool.tile([128, 128], F32, tag="pt",
                                            name=f"pt_ws{k}_{c}")
                        nc.tensor.transpose(
                            pst, ws2_sb[:, c, k * 128:(k + 1) * 128], ident)
                        nc.vector.tensor_copy(
                            out=wsT[:, k, c * 128:(c + 1) * 128], in_=pst)
                if cut < 3:
                    return None

                # step 3: sp = state @ Ws.T + bs : [64, 256]
                ps_sp = psz_pool.tile([BL, AD], F32, tag="z", name="ps_sp")
                for k in range(SD // 128):
                    nc.tensor.matmul(ps_sp, lhsT=stateT[:, k, :],
                                     rhs=wsT[:, k, :],
                                     start=(k == 0),
                                     stop=(k == SD // 128 - 1))
                sp_sb = singles.tile([BL, AD], F32)
                nc.vector.tensor_tensor(sp_sb[:], ps_sp[:], bsb_sb[:],
                                        ALU.add)
                if cut < 4:
                    return None

                # step 4: rsn = 1 / max(|sp|, eps); spbh = <sp, bh>
                sq_sp = scratch.tile([BL, AD], F32, tag="sq_sp",
                                     name="sq_sp")
                ssq_sp = singles.tile([BL, 1], F32)
                nc.scalar.activation(out=sq_sp[:], in_=sp_sb[:],
                                     func=AF.Square, accum_out=ssq_sp[:])
                # rsn = 1/max(|sp|, eps) = exp(-0.5*ln(max(ssq, eps^2)))
                # (keeps ACT on the natural_log_exp table: no table reloads)
                sn = singles.tile([BL, 1], F32)
                nc.vector.tensor_scalar_max(out=sn[:], in0=ssq_sp[:],
                                            scalar1=EPS * EPS)
                nc.scalar.activation(out=sn[:], in_=sn[:], func=AF.Ln)
                rsn = singles.tile([BL, 1], F32)
                nc.scalar.activation(out=rsn[:], in_=sn[:], func=AF.Exp,
                                     scale=-0.5)
                spbh = singles.tile([BL, 1], F32)
                dotscr = scratch.tile([BL, AD], F32, tag="sq_sp",
                                      name="dotscr")
                nc.vector.tensor_tensor(dotscr[:], sp_sb[:], bhb_sb[:],
                                        ALU.mult)
                nc.vector.reduce_sum(out=spbh[:], in_=dotscr[:], axis=AX.X)
                if cut < 5:
                    return None

                # step 5: sp.T : [128, 2, 64]
                spT = singles.tile([128, 2, BL], F32)
                for c in range(2):
                    pst = pss_pool.tile([128, BL], F32, tag="pt",
                                        name=f"pt_sp{c}")
                    nc.tensor.transpose(
                        pst, sp_sb[:, c * 128:(c + 1) * 128],
                        ident[:BL, :BL])
                    nc.vector.tensor_copy(out=spT[:, c, :], in_=pst)
                if cut < 6:
                    return None

                # step 6: q = sp @ Wh : [64, 512]
                ps_q = psz_pool.tile([BL, HD], F32, tag="z", name="ps_q")
                for c in range(2):
                    nc.tensor.matmul(ps_q, lhsT=spT[:, c, :],
                                     rhs=wh2_sb[:, c, :],
                                     start=(c == 0), stop=(c == 1))
                q_sb = singles.tile([BL, HD], F32)
                nc.vector.tensor_copy(out=q_sb[:], in_=ps_q[:])
                if cut < 7:
                    return None

                # step 7: q.T (bf16) : [128, 4, 64]
                qT = singles.tile([128, KH, BL], BF16)
                for k in range(KH):
                    pst = pss_pool.tile([128, BL], F32, tag="pt",
                                        name=f"pt_q{k}")
                    nc.tensor.transpose(pst, q_sb[:, k * 128:(k + 1) * 128],
                                        ident[:BL, :BL])
                    nc.vector.tensor_copy(out=qT[:, k, :], in_=pst)
                if cut < 8:
                    return None

                # step 8: wb = Wh.T @ bh : [128, 4]
                ps_wb = psw_pool.tile([128, KH], F32, tag="wsum",
                                      name="ps_wb")
                for k in range(KH):
                    for c in range(2):
                        nc.tensor.matmul(
                            ps_wb[:, k:k + 1],
                            lhsT=wh2_sb[:, c, k * 128:(k + 1) * 128],
                            rhs=bh2_sb[:, c:c + 1],
                            start=(c == 0), stop=(c == 1))
                if cut < 9:
                    return None

                # step 9: |bh|^2 -> broadcast [128, 1] via DRAM roundtrip
                ps_c = psw_pool.tile([1, 1], F32, tag="wsum", name="ps_c")
                for c in range(2):
                    nc.tensor.matmul(ps_c, lhsT=bh2_sb[:, c:c + 1],
                                     rhs=bh2_sb[:, c:c + 1],
                                     start=(c == 0), stop=(c == 1))
                c_sb = singles.tile([1, 1], F32)
                nc.vector.tensor_copy(out=c_sb[:], in_=ps_c[:])
                c_dram = dram.tile([1, 1], F32)
                nc.sync.dma_start(out=c_dram[:], in_=c_sb[:])
                c_bcast = singles.tile([128, 1], F32)
                nc.sync.dma_start(out=c_bcast[:],
                                  in_=c_dram.to_broadcast([128, 1]))
                if cut < 10:
                    return None

                # step 10: rsn/spbh broadcast over partitions: [128, 64, 2]
                rb_sb = singles.tile([BL, 2], F32)
                nc.vector.tensor_copy(out=rb_sb[:, 0:1], in_=rsn[:])
                nc.vector.tensor_copy(out=rb_sb[:, 1:2], in_=spbh[:])
                rb_dram = dram.tile([BL, 2], F32)
                nc.sync.dma_start(out=rb_dram[:], in_=rb_sb[:])
                rb_bcast = singles.tile([128, BL, 2], F32)
                nc.sync.dma_start(
                    out=rb_bcast[:],
                    in_=rb_dram[None].to_broadcast([128, BL, 2]))
                if cut < 11:
                    return None

                # step 11: augmented moving operand [Wh.T | q_b | wb] (bf16)
                rhs_aug = []
                for p in range(2):
                    buf = singles.tile([128, KH, AD + 2], BF16,
                                       tag=f"rhsaug{p}", name=f"rhsaug{p}")
                    rhs_aug.append(buf)
                for k in range(KH):
                    for c in range(2):
                        pst = pss_pool.tile([128, 128], F32, tag="pt",
                                            name=f"pt_wh{k}_{c}")
                        nc.tensor.transpose(
                            pst, wh2_sb[:, c, k * 128:(k + 1) * 128], ident)
                        for p in range(2):
                            nc.vector.tensor_copy(
                                out=rhs_aug[p][:, k, c * 128:(c + 1) * 128],
                                in_=pst)
                for p in range(2):
                    nc.vector.tensor_copy(
                        out=rhs_aug[p][:, :, AD + 1:AD + 2],
                        in_=ps_wb[:, :, None])
                if cut < 12:
                    return None

                # step 12: block-diagonal attn holders (bf16)
                attn_bd = []
                for p in range(2):
                    t = singles.tile([128, TPG, G], BF16, tag=f"attnbd{p}",
                                     name=f"attnbd{p}")
                    nc.vector.memset(t[:], 0.0)
                    attn_bd.append(t)

                return dict(qT=qT, rb_bcast=rb_bcast, c_bcast=c_bcast,
                            rhs_aug=rhs_aug, attn_bd=attn_bd)

            pro = _prologue()

            # ---------------- main loop ----------------
            if pro is not None and stage not in ("prologue",):
                qT = pro["qT"]
                rb_bcast = pro["rb_bcast"]
                c_bcast = pro["c_bcast"]
                rhs_aug = pro["rhs_aug"]
                attn_bd = pro["attn_bd"]

                for g in range(NG):
                    xn = xpool.tile([128, TPG, 512], BF16, tag="xnat",
                                    name=f"xn{g}")
                    nc.sync.dma_start(out=xn[:], in_=xnat[g])
                    xtt = xpool.tile([128, TPG, KH, 128], BF16, tag="xt",
                                     name=f"xt{g}")
                    nc.sync.dma_start(out=xtt[:], in_=xt[g])

                    ssq_g = work.tile([128, TPG], F32, tag="ssq",
                                      name=f"ssq{g}")
                    zqw_g = work.tile([128, TPG, 2], F32, tag="zqw",
                                      name=f"zqw{g}")

                    for bl in range(G):
                        b = g * G + bl
                        buf = rhs_aug[b % 2]
                        nc.gpsimd.tensor_copy(out=buf[:, :, AD:AD + 1],
                                              in_=qT[:, :, b:b + 1])
                        for t2 in range(2):
                            t = bl * 2 + t2
                            psz = psz_pool.tile([128, AD + 2], F32, tag="z",
                                                name=f"z{g}_{t}")
                            for k in range(KH):
                                nc.tensor.matmul(psz, lhsT=xtt[:, t, k, :],
                                                 rhs=buf[:, k, :],
                                                 start=(k == 0),
                                                 stop=(k == KH - 1))
                            sq = scratch.tile([128, AD], BF16, tag="sq",
                                              name=f"sq{g}_{t}")
                            nc.scalar.activation(out=sq[:], in_=psz[:, 0:AD],
                                                 func=AF.Square,
                                                 accum_out=ssq_g[:, t:t + 1])
                            nc.vector.tensor_copy(out=zqw_g[:, t, :],
                                                  in_=psz[:, AD:AD + 2])

                    if stage == "proj":
                        continue

                    # ---- epilogue: norms and scores ----
                    hn2 = work.tile([128, TPG], F32, tag="hn2",
                                    name=f"hn2_{g}")
                    nc.vector.tensor_scalar(
                        out=hn2[:], in0=zqw_g[:, :, 1], scalar1=2.0,
                        scalar2=c_bcast[:], op0=ALU.mult, op1=ALU.add)
                    nc.vector.tensor_tensor(hn2[:], hn2[:], ssq_g[:],
                                            ALU.add)
                    # rhn = 1/max(|hp|, eps) = exp(-0.5*ln(max(hn2, eps^2)))
                    nc.vector.tensor_scalar_max(out=hn2[:], in0=hn2[:],
                                                scalar1=EPS * EPS)
                    nc.scalar.activation(out=hn2[:], in_=hn2[:], func=AF.Ln)
                    rhn = work.tile([128, TPG], F32, tag="rhn",
                                    name=f"rhn{g}")
                    nc.scalar.activation(out=rhn[:], in_=hn2[:], func=AF.Exp,
                                         scale=-0.5)

                    scores = work.tile([128, G, 2], F32, tag="scores",
                                       name=f"scores{g}")
                    zq_v = zqw_g[:, :, 0].rearrange("p (b h) -> p b h", h=2)
                    rhn_v = rhn.rearrange("p (b h) -> p b h", h=2)
                    spbh_rep = rb_bcast[:, g * G:(g + 1) * G,
                                        1:2].to_broadcast([128, G, 2])
                    rsn_rep = rb_bcast[:, g * G:(g + 1) * G,
                                       0:1].to_broadcast([128, G, 2])
                    nc.vector.tensor_tensor(scores[:], zq_v, spbh_rep,
                                            ALU.add)
                    nc.vector.tensor_tensor(scores[:], scores[:], rsn_rep,
                                            ALU.mult)
                    nc.vector.tensor_tensor(scores[:], scores[:], rhn_v,
                                            ALU.mult)

                    if stage == "scores":
                        continue

                    # ---- softmax over N (transpose to [8, 256]) ----
                    scoresT = work.tile([G, N], F32, tag="scoresT",
                                        name=f"scT{g}")
                    for t2 in range(2):
                        pst = pss_pool.tile([G, 128], F32, tag="pt",
                                            name=f"pt_sc{g}_{t2}")
                        nc.tensor.transpose(pst, scores[:, :, t2],
                                            ident[:, :128])
                        nc.vector.tensor_copy(
                            out=scoresT[:, t2 * 128:(t2 + 1) * 128],
                            in_=pst)
                    m = work.tile([G, 1], F32, tag="m", name=f"m{g}")
                    nc.vector.reduce_max(out=m[:], in_=scoresT[:], axis=AX.X)
                    nc.vector.tensor_scalar_mul(out=m[:], in0=m[:],
                                                scalar1=-1.0)
                    e_sb = work.tile([G, N], F32, tag="esb", name=f"e{g}")
                    se = work.tile([G, 1], F32, tag="se", name=f"se{g}")
                    nc.scalar.activation(out=e_sb[:], in_=scoresT[:],
                                         func=AF.Exp, bias=m[:],
                                         accum_out=se[:])
                    nc.vector.reciprocal(out=se[:], in_=se[:])
                    attn = work.tile([G, N], F32, tag="attn",
                                     name=f"attn{g}")
                    nc.vector.tensor_scalar_mul(out=attn[:], in0=e_sb[:],
                                                scalar1=se[:])

                    if stage == "softmax":
                        continue

                    # ---- transpose attn into block-diagonal holder ----
                    abd = attn_bd[g % 2]
                    abd_flat = abd.rearrange("p t b -> p (t b)")
                    for t2 in range(2):
                        pat = pss_pool.tile([128, G], F32, tag="pt",
                                            name=f"pt_at{g}_{t2}")
                        nc.tensor.transpose(
                            pat, attn[:, t2 * 128:(t2 + 1) * 128],
                            ident[:G, :G])
                        nc.vector.tensor_copy(
                            out=abd_flat[:, t2 * G::2 * G + 1], in_=pat)

                    # ---- weighted sum over hints ----
                    psw = psw_pool.tile([G, 512], F32, tag="wsum",
                                        name=f"psw{g}")
                    for t in range(TPG):
                        nc.tensor.matmul(psw, lhsT=abd[:, t, :],
                                         rhs=xn[:, t, :],
                                         start=(t == 0), stop=(t == TPG - 1))
                    outg = work.tile([G, 512], F32, tag="outg",
                                     name=f"outg{g}")
                    nc.vector.tensor_copy(out=outg[:], in_=psw[:])
                    nc.sync.dma_start(out=out[g * G:(g + 1) * G, :],
                                      in_=outg[:])

    nc.compile()
    return nc


_NC = None


def _get_nc():
    global _NC
    if _NC is None:
        _NC = build_nc()
    return _NC


def _prep_core_inputs(state_emb, hints_emb, Ws, bs, Wh, bh, core):
    bf16 = ml_dtypes.bfloat16
    s = slice(core * BL, (core + 1) * BL)
    hf = np.ascontiguousarray(hints_emb[s]).reshape(BL * N, HD)
    hfb = hf.astype(bf16)
    # natural: (g, p, t, f) with row = g*2048 + t*128 + p
    xnat = np.ascontiguousarray(
        hfb.reshape(NG, TPG, 128, 512).transpose(0, 2, 1, 3))
    # transposed: (g, p, t, k, r) with row = g*2048 + t*128 + r, h = k*128+p
    xtd = np.ascontiguousarray(
        hfb.reshape(NG, TPG, 128, KH, 128).transpose(0, 4, 1, 3, 2))
    ws2 = np.ascontiguousarray(
        Ws.reshape(2, 128, SD).transpose(1, 0, 2)).astype(np.float32)
    wh2 = np.ascontiguousarray(
        Wh.reshape(2, 128, HD).transpose(1, 0, 2)).astype(np.float32)
    bh2 = np.ascontiguousarray(bh.reshape(2, 128).T).astype(np.float32)
    bsb = np.ascontiguousarray(
        np.broadcast_to(bs, (BL, AD))).astype(np.float32)
    bhb = np.ascontiguousarray(
        np.broadcast_to(bh, (BL, AD))).astype(np.float32)
    return {
        "state": np.ascontiguousarray(state_emb[s]).astype(np.float32),
        "xnat": xnat,
        "xt": xtd,
        "ws2": ws2,
        "bsb": bsb,
        "wh2": wh2,
        "bh2": bh2,
        "bhb": bhb,
    }


def kernel(state_emb, hints_emb, Ws, bs, Wh, bh):
    state_emb = np.asarray(state_emb, dtype=np.float32)
    hints_emb = np.asarray(hints_emb, dtype=np.float32)
    Ws = np.asarray(Ws, dtype=np.float32)
    bs = np.asarray(bs, dtype=np.float32)
    Wh = np.asarray(Wh, dtype=np.float32)
    bh = np.asarray(bh, dtype=np.float32)

    nc = _get_nc()
    in_maps = [
        _prep_core_inputs(state_emb, hints_emb, Ws, bs, Wh, bh, c)
        for c in range(NCORES)
    ]
    res = run_bass_kernel_spmd(nc, in_maps, core_ids=list(range(NCORES)))
    return np.concatenate([res.results[c]["out"] for c in range(NCORES)],
                          axis=0)
